# revision 1
# baseline (speedup 1.0000x reference)
"""Discrete VAE (VQ codebook) kernel for 8 Trainium2 NeuronCores.

Data-parallel over batch: 1024 tokens/core, 8 token-tiles of 128 tokens,
software-pipelined by emission order (scores run LOOK tiles ahead of the
scan chains; chamfer lags CLAG more rounds; min-reductions are interleaved
between the max and max_index scans of a later tile to keep Vector busy).

Per token-tile:
  scores[t,n] = sum_c x[c,t]*cb[c,n] + (-0.5*||c_n||^2)   (bf16 matmuls;
      cnorm added in-PSUM by a K=2 ones-matmul against bf16 hi+lo rows)
  PSUM -> SBUF evacuation on the Scalar engine, cast to bf16
  argmax via vector.max + max_index (1x-rate scans, the DVE floor)
  q = codebook[ids] via indirect DMA gather (bf16)
  feature-major MLP in bf16 (no inter-layer transposes):
      h1T = relu(w1@qT+b1); h2T = relu(w2@h1+b2); recT = w3@h2+b3
  chamfer: subtract/csum on GpSimd, square on Scalar, min-reductions on
      Vector into a resident per-tile buffer, summed on host.
A burst of dummy matmuls at kernel start warms the PE clock (HAM) during
the codebook DMA wait.
"""

import sys

if "/opt/trn_rl_repo" not in sys.path:
    sys.path.insert(0, "/opt/trn_rl_repo")

import os
import numpy as np
import ml_dtypes

from concourse import bacc, mybir
from concourse.bass import IndirectOffsetOnAxis
from concourse.masks import make_identity
from concourse.tile import TileContext
from concourse.bass_utils import run_bass_kernel_spmd

B, G, K, C, NT = 128, 64, 32, 256, 8192
NCORES = 8
TOK_PER_CORE = B * G // NCORES  # 1024
NTILES = TOK_PER_CORE // 128  # 8
NCHUNK = NT // 1024  # 8 psum chunks of 1024 (2 banks each)
F32 = mybir.dt.float32
BF16 = mybir.dt.bfloat16
U32 = mybir.dt.uint32
AF = mybir.ActivationFunctionType
ALU = mybir.AluOpType

_CACHE = {}


def _build():
    if "nc" in _CACHE:
        return _CACHE["nc"]

    nc = bacc.Bacc("TRN2", target_bir_lowering=False, debug=False,
                   num_devices=NCORES)

    xT = nc.dram_tensor("xT", [C, TOK_PER_CORE], BF16, kind="ExternalInput")
    cbT = nc.dram_tensor("cbT", [C, NT], BF16, kind="ExternalInput")
    cbias = nc.dram_tensor("cbias", [2, NT], BF16, kind="ExternalInput")
    cb = nc.dram_tensor("cb", [NT, C], BF16, kind="ExternalInput")
    w1T = nc.dram_tensor("w1T", [C, 512], BF16, kind="ExternalInput")
    w2T = nc.dram_tensor("w2T", [512, C], BF16, kind="ExternalInput")
    w3T = nc.dram_tensor("w3T", [C, 3 * K], BF16, kind="ExternalInput")
    b1 = nc.dram_tensor("b1", [512, 1], F32, kind="ExternalInput")
    b2 = nc.dram_tensor("b2", [C, 1], F32, kind="ExternalInput")
    b3 = nc.dram_tensor("b3", [3 * K, 1], F32, kind="ExternalInput")
    gt = nc.dram_tensor("gt", [TOK_PER_CORE, 3 * K], F32, kind="ExternalInput")
    out = nc.dram_tensor("out", [128, NTILES * 2 * K], F32, kind="ExternalOutput")

    with TileContext(nc) as tc:
        with (
            tc.tile_pool(name="const", bufs=1) as cpool,
            tc.tile_pool(name="scores", bufs=4) as spool,
            tc.tile_pool(name="work", bufs=5) as wpool,
            tc.tile_pool(name="mlp", bufs=8) as mpool,
            tc.tile_pool(name="cham", bufs=2) as chpool,
            tc.tile_pool(name="ps_score", bufs=3, space="PSUM") as ps_s,
            tc.tile_pool(name="ps_mlp", bufs=2, space="PSUM") as ps_m,
        ):
            # ---- resident constants ----
            ident = cpool.tile([128, 128], F32, tag="ident")
            make_identity(nc, ident[:])
            identb = cpool.tile([128, 128], BF16, tag="identb")
            make_identity(nc, identb[:])

            cbias_sb = cpool.tile([2, NT], BF16, tag="cbias")
            nc.sync.dma_start(out=cbias_sb[:], in_=cbias[:, :])
            ones2 = cpool.tile([2, 128], BF16, tag="ones2")
            nc.vector.memset(ones2[:], 1.0)
            cbT_sb = [cpool.tile([128, NT], BF16, tag=f"cbT{kk}",
                                 name=f"cbT_sb{kk}") for kk in range(2)]
            for ch in range(8):
                cs = slice(ch * 1024, (ch + 1) * 1024)
                for kk in range(2):
                    nc.sync.dma_start(out=cbT_sb[kk][:, cs],
                                      in_=cbT[kk * 128:(kk + 1) * 128, cs])

            w1_sb = []
            for kk in range(2):
                t = cpool.tile([128, 512], BF16, tag=f"w1_{kk}")
                nc.sync.dma_start(out=t[:], in_=w1T[kk * 128:(kk + 1) * 128, :])
                w1_sb.append(t)
            w2_sb = []
            for kk in range(4):
                t = cpool.tile([128, C], BF16, tag=f"w2_{kk}")
                nc.sync.dma_start(out=t[:], in_=w2T[kk * 128:(kk + 1) * 128, :])
                w2_sb.append(t)
            w3_sb = []
            for kk in range(2):
                t = cpool.tile([128, 3 * K], BF16, tag=f"w3_{kk}")
                nc.sync.dma_start(out=t[:], in_=w3T[kk * 128:(kk + 1) * 128, :])
                w3_sb.append(t)
            b1_sb = []
            for m in range(4):
                t = cpool.tile([128, 1], F32, tag=f"b1_{m}")
                nc.sync.dma_start(out=t[:], in_=b1[m * 128:(m + 1) * 128, :])
                b1_sb.append(t)
            b2_sb = []
            for m in range(2):
                t = cpool.tile([128, 1], F32, tag=f"b2_{m}")
                nc.sync.dma_start(out=t[:], in_=b2[m * 128:(m + 1) * 128, :])
                b2_sb.append(t)
            b3_sb = cpool.tile([3 * K, 1], F32, tag="b3")
            nc.sync.dma_start(out=b3_sb[:], in_=b3[:, :])

            mins_all = cpool.tile([128, NTILES * 2 * K], F32, tag="mins_all")

            # warm the PE (HAM) during the initial DMA wait with dummy matmuls
            warm_ps = ps_m.tile([128, 128], F32, tag="ps_mlp", name="warm_ps")
            for _ in range(22):
                nc.tensor.matmul(warm_ps[:], lhsT=ident[:], rhs=ident[:],
                                 start=True, stop=True, skip_group_check=True)

            scores_t = {}
            rec_t = {}
            dd_t = {}

            def emit_scores(t):
                ts = slice(t * 128, (t + 1) * 128)
                xt = []
                for kk in range(2):
                    x = wpool.tile([128, 128], BF16, tag="xt", name=f"xt{kk}_{t}")
                    nc.gpsimd.dma_start(
                        out=x[:], in_=xT[kk * 128:(kk + 1) * 128, ts])
                    xt.append(x)
                scores = spool.tile([128, NT], BF16, tag="scores",
                                    name=f"scores_{t}")
                scores_t[t] = scores
                GROUP = 2
                for g0 in range(0, NCHUNK, GROUP):
                    chs = list(range(g0, min(g0 + GROUP, NCHUNK)))
                    pss = {}
                    for ch in chs:
                        pss[ch] = ps_s.tile([128, 1024], F32,
                                            tag="ps_score", name=f"ps{ch}_{t}")
                    for wi, (wt, sa, so) in enumerate(
                            [(xt[0], True, False), (xt[1], False, False),
                             (ones2, False, True)]):
                        for ch in chs:
                            for half in range(2):
                                hs = slice(ch * 1024 + half * 512,
                                           ch * 1024 + (half + 1) * 512)
                                po = pss[ch][:, half * 512:(half + 1) * 512]
                                rhs = (cbT_sb[wi][:, hs] if wi < 2
                                       else cbias_sb[:, hs])
                                nc.tensor.matmul(po, lhsT=wt[:], rhs=rhs,
                                                 start=sa, stop=so)
                    for ch in chs:
                        cs = slice(ch * 1024, (ch + 1) * 1024)
                        nc.scalar.activation(out=scores[:, cs],
                                             in_=pss[ch][:], func=AF.Copy)

            def emit_mins(t):
                dd = dd_t.pop(t)
                dd3 = dd[:].rearrange("p (i j) -> p i j", j=K)
                mo = t * 2 * K
                nc.vector.tensor_reduce(out=mins_all[:, mo:mo + K], in_=dd3,
                                        axis=mybir.AxisListType.X,
                                        op=ALU.min)
                nc.vector.tensor_reduce(out=mins_all[:, mo + K:mo + 2 * K],
                                        in_=dd3.transpose([0, 2, 1]),
                                        axis=mybir.AxisListType.X,
                                        op=ALU.min)

            def emit_chain(t):
                ts = slice(t * 128, (t + 1) * 128)
                scores = scores_t.pop(t)
                max8 = wpool.tile([128, 8], BF16, tag="max8", name=f"max8_{t}")
                if t == 0:
                    cmax = wpool.tile([128, NCHUNK * 8], BF16, tag="cmax",
                                      name="cmax0")
                    for ch in range(NCHUNK):
                        nc.vector.max(out=cmax[:, ch * 8:(ch + 1) * 8],
                                      in_=scores[:, ch * 1024:(ch + 1) * 1024])
                    nc.vector.max(out=max8[:], in_=cmax[:])
                else:
                    nc.vector.max(out=max8[:], in_=scores[:])
                if t - LOOK - CLAG + 1 in dd_t:
                    emit_mins(t - LOOK - CLAG + 1)
                idx8 = wpool.tile([128, 8], U32, tag="idx8", name=f"idx8_{t}")
                nc.vector.max_index(out=idx8[:], in_max=max8[:],
                                    in_values=scores[:])

                q = wpool.tile([128, C], BF16, tag="q", name=f"q_{t}")
                nc.gpsimd.indirect_dma_start(
                    out=q[:], out_offset=None, in_=cb[:, :],
                    in_offset=IndirectOffsetOnAxis(ap=idx8[:, 0:1], axis=0),
                )

                qT = []
                for kk in range(2):
                    pt = ps_m.tile([128, 128], BF16, tag="ps_mlp",
                                   name=f"pt{kk}_{t}")
                    nc.tensor.transpose(
                        out=pt[:], in_=q[:, kk * 128:(kk + 1) * 128],
                        identity=identb[:])
                    qt = mpool.tile([128, 128], BF16, tag=f"qT{kk}",
                                    name=f"qT{kk}_{t}")
                    nc.scalar.activation(out=qt[:], in_=pt[:], func=AF.Copy)
                    qT.append(qt)

                h1 = []
                for m in range(4):
                    ph = ps_m.tile([128, 128], F32, tag="ps_mlp",
                                   name=f"ph1_{m}_{t}")
                    for kk in range(2):
                        nc.tensor.matmul(ph[:],
                                         lhsT=w1_sb[kk][:, m * 128:(m + 1) * 128],
                                         rhs=qT[kk][:],
                                         start=(kk == 0), stop=(kk == 1))
                    ht = mpool.tile([128, 128], BF16, tag="h1", name=f"h1_{m}_{t}")
                    nc.scalar.activation(out=ht[:], in_=ph[:], func=AF.Relu,
                                         bias=b1_sb[m][:])
                    h1.append(ht)

                h2 = []
                for m in range(2):
                    ph = ps_m.tile([128, 128], F32, tag="ps_mlp",
                                   name=f"ph2_{m}_{t}")
                    for kk in range(4):
                        nc.tensor.matmul(ph[:],
                                         lhsT=w2_sb[kk][:, m * 128:(m + 1) * 128],
                                         rhs=h1[kk][:],
                                         start=(kk == 0), stop=(kk == 3))
                    ht = mpool.tile([128, 128], BF16, tag="h2", name=f"h2_{m}_{t}")
                    nc.scalar.activation(out=ht[:], in_=ph[:], func=AF.Relu,
                                         bias=b2_sb[m][:])
                    h2.append(ht)

                pr = ps_m.tile([96, 128], F32, tag="ps_mlp", name=f"pr_{t}")
                for kk in range(2):
                    nc.tensor.matmul(pr[:], lhsT=w3_sb[kk][:], rhs=h2[kk][:],
                                     start=(kk == 0), stop=(kk == 1))
                recT = mpool.tile([96, 128], F32, tag="recT", name=f"recT_{t}")
                nc.scalar.activation(out=recT[:], in_=pr[:], func=AF.Identity,
                                     bias=b3_sb[:])

                prt = ps_m.tile([128, 128], F32, tag="ps_mlp", name=f"prt_{t}")
                nc.tensor.transpose(out=prt[:, 0:96], in_=recT[:],
                                    identity=ident[0:96, 0:96])
                rec = wpool.tile([128, 96], F32, tag="rec", name=f"rec_{t}")
                nc.scalar.activation(out=rec[:], in_=prt[:, 0:96], func=AF.Copy)
                rec_t[t] = rec

            def emit_cham(t):
                ts = slice(t * 128, (t + 1) * 128)
                rec = rec_t.pop(t)
                gtt = wpool.tile([128, 96], F32, tag="gt", name=f"gt_{t}")
                nc.sync.dma_start(out=gtt[:], in_=gt[ts, :])

                dif = chpool.tile([128, K * K * 3], F32, tag="dif",
                                  name=f"dif_{t}")
                rec_b = (rec[:].rearrange("p (i c) -> p i c", c=3)
                         .unsqueeze(2).broadcast_to([128, K, K, 3]))
                gt_b = (gtt[:].rearrange("p (j c) -> p j c", c=3)
                        .unsqueeze(1).broadcast_to([128, K, K, 3]))
                dif4 = dif[:].rearrange("p (i j c) -> p i j c", j=K, c=3)
                dd = chpool.tile([128, K * K], F32, tag="dd", name=f"dd_{t}")
                difc = dif[:].rearrange("p (ij c) -> p ij c", c=3)
                if t == NTILES - 1:
                    # tail tile: split by i-halves, DVE and GpSimd in parallel
                    H = K // 2
                    hd = K * K // 2  # dd elements per half
                    hf = hd * 3      # dif elements per half
                    for h, eng in ((0, nc.vector), (1, nc.gpsimd)):
                        io = slice(h * H, (h + 1) * H)
                        dslc = dif4[:, io, :, :]
                        eng.tensor_tensor(out=dslc, in0=rec_b[:, io, :, :],
                                          in1=gt_b[:, io, :, :],
                                          op=ALU.subtract)
                        fslc = dif[:, h * hf:(h + 1) * hf]
                        eng.tensor_tensor(out=fslc, in0=fslc, in1=fslc,
                                          op=ALU.mult)
                        dfc = difc[:, h * hd:(h + 1) * hd, :]
                        ddh = dd[:, h * hd:(h + 1) * hd]
                        eng.tensor_tensor(out=ddh, in0=dfc[:, :, 0],
                                          in1=dfc[:, :, 1], op=ALU.add)
                        eng.tensor_tensor(out=ddh, in0=ddh,
                                          in1=dfc[:, :, 2], op=ALU.add)
                    mo = t * 2 * K
                    dd4 = dd[:].rearrange("p (i j) -> p i j", j=K)
                    mip = chpool.tile([128, 2 * K], F32, tag="mip",
                                      name=f"mip_{t}")
                    for h in range(2):
                        io = slice(h * H, (h + 1) * H)
                        nc.vector.tensor_reduce(
                            out=mins_all[:, mo + h * H:mo + (h + 1) * H],
                            in_=dd4[:, io, :],
                            axis=mybir.AxisListType.X, op=ALU.min)
                        nc.vector.tensor_reduce(
                            out=mip[:, h * K:(h + 1) * K],
                            in_=dd4[:, io, :].transpose([0, 2, 1]),
                            axis=mybir.AxisListType.X, op=ALU.min)
                    nc.vector.tensor_tensor(out=mins_all[:, mo + K:mo + 2 * K],
                                            in0=mip[:, 0:K], in1=mip[:, K:2 * K],
                                            op=ALU.min)
                else:
                    nc.gpsimd.tensor_tensor(out=dif4, in0=rec_b, in1=gt_b,
                                            op=ALU.subtract)
                    nc.scalar.activation(out=dif[:], in_=dif[:], func=AF.Square)
                    nc.gpsimd.tensor_tensor(out=dd[:], in0=difc[:, :, 0],
                                            in1=difc[:, :, 1], op=ALU.add)
                    nc.gpsimd.tensor_tensor(out=dd[:], in0=dd[:],
                                            in1=difc[:, :, 2], op=ALU.add)
                    dd_t[t] = dd



            LOOK = 4
            CLAG = 2
            for t in range(NTILES + LOOK + CLAG):
                if LOOK <= t < NTILES + LOOK:
                    emit_chain(t - LOOK)
                if t >= LOOK + CLAG:
                    emit_cham(t - LOOK - CLAG)
                if t < NTILES:
                    emit_scores(t)

            for t in sorted(dd_t):
                emit_mins(t)

            nc.sync.dma_start(out=out[:, :], in_=mins_all[:])

    nc.compile()
    _CACHE["nc"] = nc
    return nc


def kernel(patch_features, neighborhood, codebook, w1, b1, w2, b2, w3, b3):
    nc = _build()
    bf = ml_dtypes.bfloat16

    x = np.ascontiguousarray(
        np.asarray(patch_features, np.float32).reshape(B * G, C))
    gt_full = np.ascontiguousarray(
        np.asarray(neighborhood, np.float32).reshape(B * G, 3 * K))
    cbk = np.ascontiguousarray(np.asarray(codebook, np.float32))
    cbT_h = np.ascontiguousarray(cbk.T.astype(bf))
    cn32 = (-0.5 * (cbk.astype(np.float64) ** 2).sum(1)).astype(np.float32)
    ch = cn32.astype(bf)
    cl = (cn32 - ch.astype(np.float32)).astype(bf)
    cbias_h = np.ascontiguousarray(np.stack([ch, cl]))
    w1T_h = np.ascontiguousarray(np.asarray(w1, np.float32).T.astype(bf))
    w2T_h = np.ascontiguousarray(np.asarray(w2, np.float32).T.astype(bf))
    w3T_h = np.ascontiguousarray(np.asarray(w3, np.float32).T.astype(bf))
    b1_h = np.ascontiguousarray(np.asarray(b1, np.float32).reshape(512, 1))
    b2_h = np.ascontiguousarray(np.asarray(b2, np.float32).reshape(C, 1))
    b3_h = np.ascontiguousarray(np.asarray(b3, np.float32).reshape(3 * K, 1))

    in_maps = []
    for c in range(NCORES):
        rows = slice(c * TOK_PER_CORE, (c + 1) * TOK_PER_CORE)
        in_maps.append({
            "xT": np.ascontiguousarray(x[rows].T.astype(bf)),
            "cbT": cbT_h,
            "cbias": cbias_h,
            "cb": cbk.astype(bf),
            "w1T": w1T_h, "w2T": w2T_h, "w3T": w3T_h,
            "b1": b1_h, "b2": b2_h, "b3": b3_h,
            "gt": np.ascontiguousarray(gt_full[rows]),
        })

    trace = os.environ.get("KERNEL_TRACE", "0") == "1"
    if trace:
        tmpdir = "/root/problem/_trace"
        os.makedirs(tmpdir, exist_ok=True)
        try:
            res = run_bass_kernel_spmd(nc, in_maps, list(range(NCORES)),
                                       trace=True, tmpdir=tmpdir)
        except Exception as e:
            print(f"trace run failed ({e}); retrying without trace")
            res = run_bass_kernel_spmd(nc, in_maps, list(range(NCORES)))
    else:
        res = run_bass_kernel_spmd(nc, in_maps, list(range(NCORES)))
    global LAST_EXEC_TIME_NS
    LAST_EXEC_TIME_NS = res.exec_time_ns

    total = np.float64(0.0)
    for c in range(NCORES):
        total += res.results[c]["out"].astype(np.float64).sum()
    loss = total / (B * G * K)
    return np.float32(loss)


LAST_EXEC_TIME_NS = None

